# revision 51
# baseline (speedup 1.0000x reference)
"""Causal single-head attention (B=4, S=2048, D=1024) on 8 TRN2 NeuronCores.

Sharding: 2 cores per batch; each core owns 8 q-blocks of 128 rows chosen so
both cores of a batch see the same multiset of causal kv-span lengths
(padded to 512-chunks): core h=0 -> q-blocks [0,3,4,7,8,11,12,15],
core h=1 -> [1,2,5,6,9,10,13,14]; both give span chunks [1,1,2,2,3,3,4,4].
This makes one SPMD program valid for all 8 cores; per-core differences
(which q rows, causal mask offsets) ride in the input data.

Math per core (bf16 operands, fp32 PSUM accumulation), with the host
folding M = Wq @ Wk^T / sqrt(D) so no K-projection is needed on device:
  A^T = M^T @ qT                                      (single projection)
  S_i = A_i^T.T @ kT (+ additive causal mask)         (scores vs RAW k^T)
  P = exp(S), denom = rowsum(P)                       (no max-sub: |S| < ~10)
  T_i = (P @ v) / denom                               (reassociated: raw v!)
  out_i = T_i @ Wv                                    (deferred out-proj)
Reassociation (P@v)@Wv replaces attn@(v@Wv) - saves the V projection.

All tensors ride as bf16 (host casts): halves HBM traffic vs fp32, fits
K^T, v, Wv, and the per-block T^T spill entirely in SBUF (no DRAM
round-trip), and runs PE transposes at 1.0 cycles/row. PSUM stays fp32.
Out-projection of block i-1 is interleaved behind attention of block i so
its tensor work fills dependency bubbles and the kernel has no phase-4 tail.
"""

import os

import ml_dtypes
import numpy as np

import concourse.bass as bass
import concourse.mybir as mybir
import concourse.tile as tile
from concourse import bacc
from concourse.bass_utils import run_bass_kernel_spmd

B, S, D = 4, 2048, 1024
P = 128                      # partitions / q-block rows
NBLK = 8                     # q-blocks per core
CH = 512                     # kv chunk (matmul moving free dim)
# computed kv width per q-block position: max causal span over the two cores
# of a pair (so the program stays uniform), rounded up to 128
W = [256, 512, 768, 1024, 1280, 1536, 1792, 2048]
BLOCKS = [[0, 3, 4, 7, 8, 11, 12, 15], [1, 2, 5, 6, 9, 10, 13, 14]]
DT = mybir.dt.bfloat16
F32 = mybir.dt.float32
NEG = -1e30

_cached = {}


def _build():
    if "nc" in _cached:
        return _cached["nc"]
    nc = bacc.Bacc("TRN2", target_bir_lowering=False, debug=False, num_devices=8)
    qT = nc.dram_tensor("qT", [D, P * NBLK], DT, kind="ExternalInput").ap()
    kT = nc.dram_tensor("kT", [D, S], DT, kind="ExternalInput").ap()
    v = nc.dram_tensor("v", [S, D], DT, kind="ExternalInput").ap()
    wq = nc.dram_tensor("wq", [D, D], DT, kind="ExternalInput").ap()
    wv = nc.dram_tensor("wv", [D, D], DT, kind="ExternalInput").ap()
    mask = nc.dram_tensor("mask", [P, NBLK, CH], DT, kind="ExternalInput").ap()
    ident = nc.dram_tensor("ident", [P, P], DT, kind="ExternalInput").ap()
    out = nc.dram_tensor("out", [P * NBLK, D], DT, kind="ExternalOutput").ap()

    KO = D // P      # 8 contraction chunks
    NV = S // P      # 16 v row-chunks

    kT_r = kT.rearrange("(ko p) s -> p ko s", p=P)
    v_r = v.rearrange("(so p) d -> p so d", p=P)
    wv_r = wv.rearrange("(ko p) m -> p ko m", p=P)
    wq_r = wq.rearrange("(ko p) m -> p ko m", p=P)
    qT_r = qT.rearrange("(ko p) s -> p ko s", p=P)

    with tile.TileContext(nc) as tc:
        with tc.tile_pool(name="pers", bufs=1) as pers:
            ident_sb = pers.tile([P, P], DT)
            mask_sb = pers.tile([P, NBLK, CH], DT)
            nc.sync.dma_start(ident_sb[:], ident)
            nc.sync.dma_start(mask_sb[:], mask)
            QT_sb = pers.tile([P, KO, P * NBLK], DT)
            KT_sb = pers.tile([P, KO, S], DT)
            v_sb = pers.tile([P, NV, D], DT)
            wv_sb = pers.tile([P, KO, D], DT)
            tt_sb = pers.tile([P, NBLK, KO, P], DT)

            # preload the Exp activation table under phase-1
            warm = pers.tile([P, 1], F32)
            nc.scalar.activation(warm[:], ident_sb[:, 0:1],
                                 mybir.ActivationFunctionType.Exp)

            # ---- Phase 1: A-projection (A^T = M^T qT, M folded on host);
            #      raw k^T, v, and Wv stream into SBUF underneath it ----
            with tc.tile_pool(name="ps_warm", bufs=2, space="PSUM") as pswarm, \
                 tc.tile_pool(name="ps_proj", bufs=4, space="PSUM") as psp:
                with tc.tile_pool(name="qproj", bufs=1) as qpool:
                    qT_sb = qpool.tile([P, KO, P * NBLK], DT)
                    wq_sb = qpool.tile([P, KO, D], DT)
                    # Enqueue ALL HBM loads now, in first-needed-first order:
                    # DMA start latency (~0.7us) + completion-sem propagation
                    # (~0.9us) make late enqueues expensive, and bf16 leaves
                    # enough SBUF to stage everything up front.
                    for ko in range(KO):
                        nc.sync.dma_start(wq_sb[:, ko, 0:CH],
                                          wq_r[:, ko, 0:CH])
                        nc.sync.dma_start(qT_sb[:, ko, 0:CH], qT_r[:, ko, 0:CH])
                    for ko in range(KO):
                        nc.sync.dma_start(wq_sb[:, ko, CH:D],
                                          wq_r[:, ko, CH:D])
                        nc.sync.dma_start(qT_sb[:, ko, CH:P * NBLK],
                                          qT_r[:, ko, CH:P * NBLK])
                    # k^T first half (2KB rows), v rows 0:512, then Wv early:
                    # out-proj of block 0 runs right after attention of block 1
                    for ko in range(KO):
                        nc.sync.dma_start(KT_sb[:, ko, 0:2 * CH],
                                          kT_r[:, ko, 0:2 * CH])
                    for so in range(4):
                        nc.sync.dma_start(v_sb[:, so], v_r[:, so])
                    for ko in range(KO):
                        nc.sync.dma_start(wv_sb[:, ko], wv_r[:, ko])
                    for so in range(4, 8):
                        nc.sync.dma_start(v_sb[:, so], v_r[:, so])
                    for ko in range(KO):
                        nc.sync.dma_start(KT_sb[:, ko, 2 * CH:S],
                                          kT_r[:, ko, 2 * CH:S])
                    for so in range(8, NV):
                        nc.sync.dma_start(v_sb[:, so], v_r[:, so])
                    # PE p-state ramp: a few dummy transposes wake the array
                    # while the first wq/qT chunks land
                    for _ in range(36):
                        wps = pswarm.tile([P, P], DT, tag="w")
                        nc.tensor.transpose(wps[:], ident_sb[:], ident_sb[:])
                    for n in range(2):
                        for m in range(NBLK):
                            ps = psp.tile([P, CH], F32, tag="pp")
                            for k in range(KO):
                                nc.tensor.matmul(
                                    ps[:], wq_sb[:, k, bass.ts(m, P)],
                                    qT_sb[:, k, bass.ts(n, CH)],
                                    start=(k == 0), stop=(k == KO - 1))
                            nc.vector.tensor_copy(QT_sb[:, m, bass.ts(n, CH)],
                                                  ps[:])

            # ---- Phase 3: attention per q-block, pipelined: scores run one
            #      chunk ahead of transpose+AV. Out-proj of block i-1 rides
            #      behind attention of block i (phase 4 interleaved). ----
            with tc.tile_pool(name="cwork", bufs=2) as cwork, \
                 tc.tile_pool(name="ppool", bufs=3) as ppool, \
                 tc.tile_pool(name="ptpool", bufs=4) as ptpool, \
                 tc.tile_pool(name="owork", bufs=2) as owork, \
                 tc.tile_pool(name="ps_s", bufs=2, space="PSUM") as ps_s, \
                 tc.tile_pool(name="ps_tr", bufs=2, space="PSUM") as ps_tr, \
                 tc.tile_pool(name="ps_t", bufs=1, space="PSUM") as ps_t, \
                 tc.tile_pool(name="ps_o", bufs=1, space="PSUM") as ps_o:

                rdens = {}

                def attention_block(i):
                    wi = W[i]
                    nch = (wi + CH - 1) // CH
                    nkv = wi // P
                    ps_T0 = ps_t.tile([P, CH], F32, tag="T0", name=f"T0_{i}")
                    ps_T1 = ps_t.tile([P, CH], F32, tag="T1", name=f"T1_{i}")
                    dsums = []
                    p_tiles = []

                    def emit_scores(c, i=i, nch=nch, wi=wi):
                        w = min(CH, wi - c * CH)
                        ps_c = ps_s.tile([P, CH], F32, tag="s",
                                         name=f"s_{i}_{c}")
                        for k in range(KO):
                            nc.tensor.matmul(
                                ps_c[:, 0:w], QT_sb[:, k, bass.ts(i, P)],
                                KT_sb[:, k, bass.ds(c * CH, w)],
                                start=(k == 0), stop=(k == KO - 1))
                        if c == nch - 1:
                            nc.vector.tensor_tensor(
                                ps_c[:, 0:w], ps_c[:, 0:w],
                                mask_sb[:, i, 0:w], mybir.AluOpType.add)
                        p_sb = ppool.tile([P, CH], DT, tag="p",
                                          name=f"p_{i}_{c}")
                        ds = cwork.tile([P, 1], F32, tag="ds",
                                        name=f"ds_{i}_{c}")
                        nc.scalar.activation(
                            p_sb[:, 0:w], ps_c[:, 0:w],
                            mybir.ActivationFunctionType.Exp, accum_out=ds[:])
                        dsums.append(ds)
                        p_tiles.append(p_sb)

                    def emit_trav(c, i=i, nkv=nkv, wi=wi):
                        # transposes run 2 ahead of the AV matmuls
                        nt = min(CH, wi - c * CH) // P
                        pts = []
                        for t in range(nt):
                            ptr = ps_tr.tile([P, P], DT, tag="tr")
                            nc.tensor.transpose(
                                ptr[:], p_tiles[c][:, bass.ts(t, P)],
                                ident_sb[:])
                            pt_sb = ptpool.tile([P, P], DT, tag="pt")
                            nc.scalar.activation(
                                pt_sb[:], ptr[:],
                                mybir.ActivationFunctionType.Copy)
                            pts.append(pt_sb)
                            if t >= 2:
                                _emit_av(c, t - 2, pts[t - 2], i, nkv)
                        for t in range(max(0, nt - 2), nt):
                            _emit_av(c, t, pts[t], i, nkv)

                    def _emit_av(c, t, pt_sb, i, nkv):
                        kvi = c * (CH // P) + t
                        nc.tensor.matmul(
                            ps_T0[:], pt_sb[:], v_sb[:, kvi, 0:CH],
                            start=(kvi == 0), stop=(kvi == nkv - 1))
                        nc.tensor.matmul(
                            ps_T1[:], pt_sb[:], v_sb[:, kvi, CH:D],
                            start=(kvi == 0), stop=(kvi == nkv - 1))

                    for c in range(nch):
                        emit_scores(c)
                        if c >= 1:
                            emit_trav(c - 1)
                    emit_trav(nch - 1)

                    denom = cwork.tile([P, 1], F32, tag="den")
                    if nch == 1:
                        nc.gpsimd.tensor_copy(denom[:], dsums[0][:])
                    else:
                        nc.gpsimd.tensor_tensor(
                            denom[:], dsums[0][:], dsums[1][:],
                            mybir.AluOpType.add)
                        for c in range(2, nch):
                            nc.gpsimd.tensor_tensor(
                                denom[:], denom[:], dsums[c][:],
                                mybir.AluOpType.add)
                    rden = cwork.tile([P, 1], F32, tag="rden")
                    nc.vector.reciprocal(rden[:], denom[:])
                    rdens[i] = rden
                    # T stays UNnormalized: the plain cast needs no denom,
                    # so the block tail doesn't wait on the exp-accum chain.
                    # 1/denom is applied per q-row in the out-proj store.
                    t_st = cwork.tile([P, D], DT, tag="tst", bufs=1)
                    nc.vector.tensor_copy(t_st[:, 0:CH], ps_T0[:])
                    nc.vector.tensor_copy(t_st[:, CH:D], ps_T1[:])
                    for d in range(KO):
                        ptr = ps_tr.tile([P, P], DT, tag="tr")
                        nc.tensor.transpose(
                            ptr[:], t_st[:, bass.ts(d, P)], ident_sb[:])
                        nc.scalar.activation(
                            tt_sb[:, i, d], ptr[:],
                            mybir.ActivationFunctionType.Copy)

                def _out_store(i, ps_o0, ps_o1):
                    # CASTs stay on DVE: a PSUM-gated op on the Scalar queue
                    # would head-of-line block the next attention block's exp.
                    # The deferred 1/denom rides along as a per-row scale.
                    o_sb = owork.tile([P, D], DT, tag="osb")
                    nc.vector.tensor_scalar_mul(o_sb[:, 0:CH], ps_o0[:],
                                                rdens[i][:])
                    nc.vector.tensor_scalar_mul(o_sb[:, CH:D], ps_o1[:],
                                                rdens[i][:])
                    nc.sync.dma_start(out[bass.ts(i, P), 0:CH],
                                      o_sb[:, 0:CH])
                    nc.sync.dma_start(out[bass.ts(i, P), CH:D],
                                      o_sb[:, CH:D])

                def out_proj(i):
                    ps_o0 = ps_o.tile([P, CH], F32, tag="o0", name=f"o0_{i}")
                    ps_o1 = ps_o.tile([P, CH], F32, tag="o1", name=f"o1_{i}")
                    for d in range(KO):
                        nc.tensor.matmul(
                            ps_o0[:], tt_sb[:, i, d], wv_sb[:, d, 0:CH],
                            start=(d == 0), stop=(d == KO - 1))
                        nc.tensor.matmul(
                            ps_o1[:], tt_sb[:, i, d], wv_sb[:, d, CH:D],
                            start=(d == 0), stop=(d == KO - 1))
                    _out_store(i, ps_o0, ps_o1)

                for i in range(NBLK):
                    attention_block(i)
                    if i >= 1:
                        out_proj(i - 1)
                out_proj(NBLK - 1)

    nc.compile()
    _cached["nc"] = nc
    return nc


LAST_RESULT = None


def kernel(q, k, v, Wq, Wk, Wv, mask):
    global LAST_RESULT
    q = np.asarray(q, dtype=np.float32)
    k = np.asarray(k, dtype=np.float32)
    v = np.asarray(v, dtype=np.float32)
    Wq = np.asarray(Wq, dtype=np.float32)
    Wk = np.asarray(Wk, dtype=np.float32)
    Wv = np.asarray(Wv, dtype=np.float32)

    nc = _build()

    bf16 = ml_dtypes.bfloat16
    wm = np.ascontiguousarray(
        (Wq.astype(np.float64) @ Wk.astype(np.float64).T
         / np.sqrt(np.float64(D))).astype(bf16))
    wv_c = np.ascontiguousarray(Wv.astype(bf16))
    ident = np.eye(P, dtype=bf16)

    masks = []
    r = np.arange(P)[:, None]
    c = np.arange(CH)[None, :]
    for h in range(2):
        m = np.zeros((P, NBLK, CH), dtype=np.float32)
        for i in range(NBLK):
            j = BLOCKS[h][i]
            q0 = P * j
            nch = (W[i] + CH - 1) // CH
            last_off = CH * (nch - 1)
            w_last = W[i] - last_off
            mi = np.where(last_off + c <= q0 + r, 0.0, NEG)
            mi[:, w_last:] = 0.0
            m[:, i, :] = mi
        masks.append(m.astype(bf16))

    in_maps = []
    for core in range(8):
        b, h = core // 2, core % 2
        blocks = BLOCKS[h]
        qTb = q[b].T  # [D, S]
        cols = np.concatenate([np.arange(j * P, (j + 1) * P) for j in blocks])
        in_maps.append({
            "qT": np.ascontiguousarray(qTb[:, cols].astype(bf16)),
            "kT": np.ascontiguousarray(k[b].T.astype(bf16)),
            "v": np.ascontiguousarray(v[b].astype(bf16)),
            "wq": wm, "wv": wv_c,
            "mask": masks[h], "ident": ident,
        })

    res = run_bass_kernel_spmd(nc, in_maps, list(range(8)),
                               trace=bool(os.environ.get("KERNEL_TRACE")))
    LAST_RESULT = res

    out = np.empty((B, S, D), dtype=np.float32)
    for core in range(8):
        b, h = core // 2, core % 2
        oc = np.asarray(res.results[core]["out"], dtype=np.float32)
        for pos, j in enumerate(BLOCKS[h]):
            out[b, j * P:(j + 1) * P, :] = oc[pos * P:(pos + 1) * P, :]
    return out
